# revision 15
# baseline (speedup 1.0000x reference)
"""Bidirectional Mamba encoder layer on 8 Trainium2 NeuronCores.

Sharding: 8-way data parallel over (batch=4) x (seq halves=2). Each core
processes one (b, half) slice with a HALO-token warm-up window on each side
so the selective scans start from a decayed (effectively exact) state.
No collectives needed.

Per-core dataflow (channel-major):
  LN1 (token-major) -> transpose -> h_T
  per direction d in (fwd, bwd):          # bwd handled by storing reversed time
    inproj matmul -> xin_T (ext, +3 zero pad), z (silu, main only)
    depthwise conv (DVE shifts) + silu -> xc_T
    xp-proj -> dt_r, B, C;  dt-proj -> softplus (Exp+Ln) -> dt; dtx = dt*xc
    scan: for n in 16: broadcast B_n/C_n rows to 128 partitions (DMA);
          for cb in 8: dA = Exp(A[:,n]*dt) (ACT, per-partition scale);
                       dBx = dtx * B_rep; h = tensor_tensor_scan(dA, dBx);
                       y_acc += h * C_rep (main window only)
    gate: (y_acc + xc*dparam) * silu(z); outproj matmul (f stored, b added rev)
  residual + LN2 (channel-major, mean/var via ones-matmul) + FFN (gelu tanh)
Output: out^T [512, 1024] per core; host reassembles.
"""
import os
import sys
import numpy as np
from contextlib import ExitStack

for _p in ("/opt/trn_rl_repo", "/opt/pypackages"):
    if _p not in sys.path and os.path.isdir(_p):
        sys.path.append(_p)

import concourse.bass as bass
import concourse.mybir as mybir
import concourse.tile as tile

F32 = mybir.dt.float32
F16 = mybir.dt.float16
AF = mybir.ActivationFunctionType
OP = mybir.AluOpType


# ---------------------------------------------------------------------------
# This walrus build accepts at most ONE sync-wait command per instruction.
# Tile emits several; hoist all but one onto same-engine NoOps (engines
# dispatch in program order, so this is semantically identical).
import json as _json


def _split_waits(bir_json_bytes, max_waits=1):
    d = _json.loads(bir_json_bytes)
    uid = [0]
    for fn in d.get("functions", []):
        for blk in fn.get("blocks", []):
            ins_list = blk.get("instructions")
            if not ins_list:
                continue
            out = []
            for ins in ins_list:
                si = ins.get("sync_info") or {}
                waits = si.get("on_wait") or []
                if len(waits) > max_waits:
                    keep = waits[-max_waits:]
                    for w in waits[:-max_waits]:
                        uid[0] += 1
                        out.append({
                            "name": f"{ins['name']}-sw{uid[0]}",
                            "opcode": "NoOp",
                            "engine": ins["engine"],
                            "ins": [],
                            "outs": [],
                            "sync_info": {"on_wait": [w]},
                        })
                    si["on_wait"] = keep
                    ins["sync_info"] = si
                out.append(ins)
            blk["instructions"] = out
    return _json.dumps(d).encode()


def _install_split_waits():
    if getattr(mybir, "_ant_split_waits_installed", False):
        return
    _orig = mybir.module_to_json_bytes

    def _patched(m, *a, **k):
        return _split_waits(_orig(m, *a, **k))

    mybir.module_to_json_bytes = _patched
    mybir._ant_split_waits_installed = True


_install_split_waits()

D_MODEL = 512
D_INNER = 1024
D_STATE = 16
D_CONV = 4
DT_RANK = 32
D_FF = 2048
B, L = 4, 2048
EPS = 1e-5

H = 64                 # halo tokens on each side of the main window
LM = 1024              # main tokens per core
LEXT = LM + 2 * H      # 1152
NCB = D_INNER // 128   # 8 channel blocks
NMT_DM = D_MODEL // 128  # 4 d_model tiles
NMT_FF = D_FF // 128     # 16 ff tiles
NKC_DM = D_MODEL // 128  # 4 k-chunks for d_model contraction

MAIN_LO, MAIN_HI = H, H + LM

W32 = 38 + 2 * 184          # consolidated fp32 consts width
W16 = 129 + 2 * D_MODEL + LM  # ident | ones | n1g | n1b | keep_rep


def _chunks(total, size=512):
    return [(i, min(size, total - i)) for i in range(0, total, size)]

TCH = _chunks(LEXT)     # time chunks for ext-width matmuls
MCH = _chunks(LM)       # time chunks for main-width matmuls


def _rev_free(ap):
    """Reverse the innermost (free) axis of a 2D AP view."""
    (pstep, pcnt), (fstep, fcnt) = ap.ap[0], ap.ap[1]
    return bass.AP(
        tensor=ap.tensor,
        offset=ap.offset + (fcnt - 1) * fstep,
        ap=[[pstep, pcnt], [-fstep, fcnt]],
    )


def _bcast_rows(dram, row0, nrows, width, nparts=128):
    """AP reading `nrows` DRAM rows, each broadcast across nparts partitions."""
    base = dram[row0:row0 + nrows, 0:width]
    return bass.AP(
        tensor=base.tensor,
        offset=base.offset,
        ap=[[0, nparts], [width, nrows], [1, width]],
    )


def _bcast_row(dram, row, width, nparts=128):
    """AP reading one DRAM row broadcast across nparts partitions."""
    base = dram[row:row + 1, 0:width]
    return bass.AP(
        tensor=base.tensor,
        offset=base.offset,
        ap=[[0, nparts], [1, width]],
    )


def build_program():
    nc = bass.Bass("TRN2", target_bir_lowering=False, debug=False)

    io = {}
    def din(name, shape, dt=F32):
        io[name] = nc.dram_tensor(name, shape, dt, kind="ExternalInput").ap()
        return io[name]

    din("x_ext", [LEXT, D_MODEL])
    din("x_mainT", [D_MODEL, LM])
    din("cst32", [128, W32])
    din("cst16", [128, W16], F16)
    for d in ("f", "b"):
        din("inw_" + d, [D_MODEL, 2 * D_INNER], F16)
        din("xpw_" + d, [D_INNER, DT_RANK + 2 * D_STATE], F16)
        din("dtw_" + d, [DT_RANK, D_INNER], F16)
        din("outw_" + d, [D_INNER, D_MODEL], F16)
    din("ffn_w1", [D_MODEL, D_FF], F16)
    din("ffn_w2", [D_FF, D_MODEL], F16)

    outT = nc.dram_tensor("outT", [D_MODEL, LM], F32, kind="ExternalOutput").ap()
    bc_scr = {d: nc.dram_tensor(f"bc_scr_{d}", [2 * D_STATE, LEXT], F16,
                                kind="Internal").ap() for d in ("f", "b")}
    mr_scr = nc.dram_tensor("mr_scr", [2, LM], F32, kind="Internal").ap()

    with tile.TileContext(nc) as tc, ExitStack() as ctx:
        _emit(ctx, tc, nc, io, outT, bc_scr, mr_scr)
    return nc


def _emit(ctx, tc, nc, io, outT, bc_scr, mr_scr):
    psum = ctx.enter_context(tc.tile_pool(name="psum", bufs=4, space="PSUM"))
    psum_tp = ctx.enter_context(tc.tile_pool(name="psum_tp", bufs=2, space="PSUM"))
    const = ctx.enter_context(tc.tile_pool(name="const", bufs=1))

    # ---------------- constants: two consolidated blocks ----------------
    c32 = const.tile([128, W32], F32, name="c32", tag="c32")
    nc.gpsimd.dma_start(out=c32[:, :], in_=io["cst32"][:, :])
    c16 = const.tile([128, W16], F16, name="c16", tag="c16")
    nc.gpsimd.dma_start(out=c16[:, :], in_=io["cst16"][:, :])

    eps_t = c32[:, 0:1]
    def keep_tok(i):
        return c32[:, 1 + i:2 + i]
    def n2g_c(m):
        return c32[:, 10 + m:11 + m]
    def n2b_c(m):
        return c32[:, 14 + m:15 + m]
    def fb2_c(m):
        return c32[:, 18 + m:19 + m]
    def fb1_c(mt):
        return c32[:, 22 + mt:23 + mt]
    def _db(d):
        return 38 + (0 if d == "f" else 184)
    def convw_c(d, c, k):
        o = _db(d) + c * 4 + k
        return c32[:, o:o + 1]
    def convb_c(d, c):
        o = _db(d) + 32 + c
        return c32[:, o:o + 1]
    def dtb_c(d, c):
        o = _db(d) + 40 + c
        return c32[:, o:o + 1]
    def dparam_c(d, c):
        o = _db(d) + 48 + c
        return c32[:, o:o + 1]
    def A_c(d, c, n):
        o = _db(d) + 56 + c * 16 + n
        return c32[:, o:o + 1]

    ident = c16[:, 0:128]
    ones16 = c16[:, 128:129]
    n1g_rep = c16[:, 129:129 + D_MODEL]
    n1b_rep = c16[:, 129 + D_MODEL:129 + 2 * D_MODEL]
    keep_rep = c16[:, 129 + 2 * D_MODEL:129 + 2 * D_MODEL + LM]

    # ---------------- Phase 1: LN1 + transpose -> h_T ----------------
    hT_pool = ctx.enter_context(tc.tile_pool(name="hT_pool", bufs=1))
    hT = [hT_pool.tile([128, LEXT], F16, name=f"hT{j}", tag=f"hT{j}")
          for j in range(NKC_DM)]

    with tc.tile_pool(name="p1", bufs=3) as p1:
        x_tiled = io["x_ext"].rearrange("(n p) d -> n p d", p=128)
        for i in range(LEXT // 128):
            xt = p1.tile([128, D_MODEL], F32, name="xt", tag="xt", bufs=9)
            nc.gpsimd.dma_start(out=xt[:, :], in_=x_tiled[i])
            stats = p1.tile([128, 6], F32, name="stats", tag="stats")
            nc.vector.bn_stats(out=stats[:, :], in_=xt[:, :])
            mv = p1.tile([128, 2], F32, name="mv", tag="mv")
            nc.vector.bn_aggr(out=mv[:, :], in_=stats[:, :])
            nc.scalar.activation(out=mv[:, 1:2], in_=mv[:, 1:2], func=AF.Sqrt,
                                 bias=eps_t, scale=1.0)
            nc.vector.reciprocal(out=mv[:, 1:2], in_=mv[:, 1:2])
            xn = p1.tile([128, D_MODEL], F32, name="xn", tag="xn")
            nc.vector.tensor_scalar(out=xn[:, :], in0=xt[:, :],
                                    scalar1=mv[:, 0:1], scalar2=mv[:, 1:2],
                                    op0=OP.subtract, op1=OP.mult)
            tmp = p1.tile([128, D_MODEL], F32, name="tmp", tag="tmp")
            nc.vector.scalar_tensor_tensor(out=tmp[:, :], in0=xn[:, :],
                                           scalar=keep_tok(i),
                                           in1=n1g_rep[:, :],
                                           op0=OP.mult, op1=OP.mult)
            htok = p1.tile([128, D_MODEL], F16, name="htok", tag="htok")
            nc.vector.scalar_tensor_tensor(out=htok[:, :], in0=n1b_rep[:, :],
                                           scalar=keep_tok(i), in1=tmp[:, :],
                                           op0=OP.mult, op1=OP.add)
            for j in range(NKC_DM):
                ps = psum_tp.tile([128, 128], F16, name="tp", tag="tp")
                nc.tensor.transpose(ps[:, :], htok[:, 128 * j:128 * (j + 1)],
                                    ident)
                nc.vector.tensor_copy(out=hT[j][:, 128 * i:128 * (i + 1)],
                                      in_=ps[:, :])

    # ---------------- per-direction pipeline ----------------
    ym_pool = ctx.enter_context(tc.tile_pool(name="ym_pool", bufs=1))
    ym = [ym_pool.tile([128, LM], F16, name=f"ym{m}", tag=f"ym{m}")
          for m in range(NMT_DM)]
    wpool = ctx.enter_context(tc.tile_pool(name="wpool", bufs=1))

    for d in ("f", "b"):
        rev = (d == "b")
        with tc.tile_pool(name="dirp" + d, bufs=1) as dp, \
             tc.tile_pool(name="work" + d, bufs=2) as wk:
            inw_t = [wpool.tile([128, 2 * D_INNER], F16, name=f"inw{k}",
                                tag=f"inw{k}") for k in range(NKC_DM)]
            inw_r = io["inw_" + d].rearrange("(n p) c -> n p c", p=128)
            for k in range(NKC_DM):
                nc.gpsimd.dma_start(out=inw_t[k][:, :], in_=inw_r[k])

            # big tag group: xin / xc / dt / dtx share recycled slots
            def big(nm):
                return dp.tile([128, 3 + LEXT], F16, name=nm, tag="big", bufs=17)

            xin = [big(f"xin{c}") for c in range(NCB)]
            z_s = [dp.tile([128, LM], F16, name=f"z{c}", tag="z", bufs=NCB)
                   for c in range(NCB)]
            for c in range(NCB):
                nc.vector.memset(xin[c][:, 0:3], 0.0)

            # ---- inproj ----
            for mt in range(2 * NCB):
                for (nlo, ncnt) in TCH:
                    ps = psum.tile([128, 512], F32, name="mm", tag="mm")
                    for k in range(NKC_DM):
                        nc.tensor.matmul(ps[:, 0:ncnt],
                                         inw_t[k][:, 128 * mt:128 * (mt + 1)],
                                         hT[k][:, nlo:nlo + ncnt],
                                         start=(k == 0), stop=(k == NKC_DM - 1))
                    if mt < NCB:  # xin part
                        if not rev:
                            nc.vector.tensor_copy(
                                out=xin[mt][:, 3 + nlo:3 + nlo + ncnt],
                                in_=ps[:, 0:ncnt])
                        else:
                            nc.vector.tensor_copy(
                                out=xin[mt][:, 3 + LEXT - nlo - ncnt:3 + LEXT - nlo],
                                in_=_rev_free(ps[:, 0:ncnt]))
                    else:       # z part -> silu, main window only
                        c = mt - NCB
                        lo, hi = max(MAIN_LO, nlo), min(MAIN_HI, nlo + ncnt)
                        if lo >= hi:
                            continue
                        zr = wk.tile([128, 512], F16, name="zraw", tag="zraw")
                        zg = wk.tile([128, 512], F16, name="zsig", tag="zsig")
                        if not rev:
                            nc.vector.tensor_copy(out=zr[:, 0:hi - lo],
                                                  in_=ps[:, lo - nlo:hi - nlo])
                        else:
                            nc.vector.tensor_copy(
                                out=zr[:, 0:hi - lo],
                                in_=_rev_free(ps[:, lo - nlo:hi - nlo]))
                        nc.scalar.activation(out=zg[:, 0:hi - lo],
                                             in_=zr[:, 0:hi - lo], func=AF.Sigmoid)
                        o0 = (lo - MAIN_LO) if not rev else (MAIN_HI - hi)
                        nc.vector.tensor_tensor(out=z_s[c][:, o0:o0 + hi - lo],
                                                in0=zr[:, 0:hi - lo],
                                                in1=zg[:, 0:hi - lo], op=OP.mult)

            # ---- conv + silu -> xc ----
            xc = [big(f"xc{c}") for c in range(NCB)]
            for c in range(NCB):
                acc = wk.tile([128, LEXT], F16, name="cacc", tag="cacc")
                nc.vector.tensor_scalar(out=acc[:, :], in0=xin[c][:, 0:LEXT],
                                        scalar1=convw_c(d, c, 0),
                                        scalar2=convb_c(d, c),
                                        op0=OP.mult, op1=OP.add)
                for k in range(1, D_CONV):
                    nc.vector.scalar_tensor_tensor(
                        out=acc[:, :], in0=xin[c][:, k:k + LEXT],
                        scalar=convw_c(d, c, k),
                        in1=acc[:, :], op0=OP.mult, op1=OP.add)
                sg = wk.tile([128, LEXT], F16, name="csig", tag="csig")
                nc.scalar.activation(out=sg[:, :], in_=acc[:, :], func=AF.Sigmoid)
                nc.vector.tensor_tensor(out=xc[c][:, 0:LEXT], in0=acc[:, :],
                                        in1=sg[:, :], op=OP.mult)

            # ---- xp proj -> dt_r, B, C ----
            xpw_t = wpool.tile([128, NCB, 2 * D_STATE + DT_RANK], F16,
                               name="xpw_t", tag="xpw_t")
            nc.gpsimd.dma_start(out=xpw_t[:, :, :],
                              in_=io["xpw_" + d].rearrange("(n p) w -> p n w",
                                                           p=128))
            dtr = dp.tile([DT_RANK, LEXT], F16, name="dtr", tag="dtr")
            bc_sb = wk.tile([2 * D_STATE, LEXT], F16, name="bc_sb", tag="bc_sb")
            for (nlo, ncnt) in TCH:
                ps = psum.tile([64, 512], F32, name="xp", tag="mm")
                for k in range(NCB):
                    nc.tensor.matmul(ps[:, 0:ncnt], xpw_t[:, k, :],
                                     xc[k][:, nlo:nlo + ncnt],
                                     start=(k == 0), stop=(k == NCB - 1))
                nc.vector.tensor_copy(out=dtr[:, nlo:nlo + ncnt],
                                      in_=ps[0:DT_RANK, 0:ncnt])
                nc.vector.tensor_copy(out=bc_sb[:, nlo:nlo + ncnt],
                                      in_=ps[DT_RANK:DT_RANK + 2 * D_STATE, 0:ncnt])
            nc.gpsimd.dma_start(out=bc_scr[d][:, :], in_=bc_sb[:, :])

            # ---- dt proj + softplus + dtx + xc main copy ----
            dtw_t = wpool.tile([DT_RANK, D_INNER], F16, name="dtw_t", tag="dtw_t")
            nc.gpsimd.dma_start(out=dtw_t[:, :], in_=io["dtw_" + d][:, :])
            dt_t, dtx, xcm = [], [], []
            for c in range(NCB):
                dt_c = big(f"dt{c}")
                for (nlo, ncnt) in TCH:
                    ps = psum.tile([128, 512], F32, name="mm", tag="mm")
                    nc.tensor.matmul(ps[:, 0:ncnt],
                                     dtw_t[:, 128 * c:128 * (c + 1)],
                                     dtr[:, nlo:nlo + ncnt], start=True, stop=True)
                    # softplus(x) = ln(1 + exp(x)); same ACT set as scan's Exp
                    nc.scalar.activation(out=ps[:, 0:ncnt], in_=ps[:, 0:ncnt],
                                         func=AF.Exp,
                                         bias=dtb_c(d, c), scale=1.0)
                    nc.scalar.activation(out=dt_c[:, nlo:nlo + ncnt],
                                         in_=ps[:, 0:ncnt], func=AF.Ln,
                                         bias=1.0, scale=1.0)
                dtx_c = big(f"dtx{c}")
                nc.vector.tensor_tensor(out=dtx_c[:, 0:LEXT], in0=dt_c[:, 0:LEXT],
                                        in1=xc[c][:, 0:LEXT], op=OP.mult)
                xcm_c = dp.tile([128, LM], F16, name=f"xcm{c}", tag="xcm",
                                bufs=NCB)
                nc.vector.tensor_copy(out=xcm_c[:, :],
                                      in_=xc[c][:, MAIN_LO:MAIN_HI])
                dt_t.append(dt_c); dtx.append(dtx_c); xcm.append(xcm_c)

            # ---- scan over states ----
            yacc = [dp.tile([128, LM], F16, name=f"ya{c}", tag="yg",
                            bufs=NCB + 1) for c in range(NCB)]
            with tc.tile_pool(name="scan" + d, bufs=2) as sp, \
                 tc.tile_pool(name="bcrep" + d, bufs=2) as bp:
                for g in range(D_STATE // 2):
                    b_grp = bp.tile([128, 2, LEXT], F16, name="b_grp", tag="b_grp")
                    c_grp = bp.tile([128, 2, LEXT], F16, name="c_grp", tag="c_grp")
                    nc.gpsimd.dma_start(out=b_grp[:, :, :],
                                        in_=_bcast_rows(bc_scr[d], 2 * g, 2, LEXT))
                    nc.gpsimd.dma_start(
                        out=c_grp[:, :, :],
                        in_=_bcast_rows(bc_scr[d], D_STATE + 2 * g, 2, LEXT))
                    for nn in range(2):
                        n = 2 * g + nn
                        b_rep = b_grp[:, nn, :]
                        c_rep = c_grp[:, nn, :]
                        for c in range(NCB):
                            dA = sp.tile([128, LEXT], F16, name="dA", tag="dA")
                            nc.scalar.activation(out=dA[:, :],
                                                 in_=dt_t[c][:, 0:LEXT],
                                                 func=AF.Exp,
                                                 scale=A_c(d, c, n))
                            dBx = sp.tile([128, LEXT], F16, name="dBx", tag="dBx")
                            nc.vector.tensor_tensor(out=dBx[:, :],
                                                    in0=dtx[c][:, 0:LEXT],
                                                    in1=b_rep, op=OP.mult)
                            hsc = sp.tile([128, LEXT], F16, name="hsc", tag="hsc")
                            nc.vector.tensor_tensor_scan(
                                out=hsc[:, :], data0=dA[:, :], data1=dBx[:, :],
                                initial=0.0, op0=OP.mult, op1=OP.add)
                            if n == 0:
                                nc.vector.tensor_tensor(
                                    out=yacc[c][:, :],
                                    in0=hsc[:, MAIN_LO:MAIN_HI],
                                    in1=c_rep[:, MAIN_LO:MAIN_HI], op=OP.mult)
                            else:
                                hc = sp.tile([128, LM], F16, name="hc", tag="hc")
                                nc.vector.tensor_tensor(
                                    out=hc[:, :], in0=hsc[:, MAIN_LO:MAIN_HI],
                                    in1=c_rep[:, MAIN_LO:MAIN_HI], op=OP.mult)
                                nc.vector.tensor_tensor(out=yacc[c][:, :],
                                                        in0=yacc[c][:, :],
                                                        in1=hc[:, :], op=OP.add)

            # ---- gate ----
            g_t = []
            for c in range(NCB):
                t1 = wk.tile([128, LM], F16, name="gate1", tag="gate1")
                nc.vector.scalar_tensor_tensor(
                    out=t1[:, :], in0=xcm[c][:, :],
                    scalar=dparam_c(d, c), in1=yacc[c][:, :],
                    op0=OP.mult, op1=OP.add)
                g_c = dp.tile([128, LM], F16, name=f"g{c}", tag="yg",
                              bufs=NCB + 1)
                nc.vector.tensor_tensor(out=g_c[:, :], in0=t1[:, :],
                                        in1=z_s[c][:, :], op=OP.mult)
                g_t.append(g_c)

            # ---- outproj (f: copy, b: add reversed) ----
            outw_t = wpool.tile([128, NCB, D_MODEL], F16, name="outw_t",
                                tag="outw_t")
            nc.gpsimd.dma_start(out=outw_t[:, :, :],
                              in_=io["outw_" + d].rearrange("(n p) w -> p n w",
                                                            p=128))
            for m in range(NMT_DM):
                for (nlo, ncnt) in MCH:
                    ps = psum.tile([128, 512], F32, name="mm", tag="mm")
                    for k in range(NCB):
                        nc.tensor.matmul(ps[:, 0:ncnt],
                                         outw_t[:, k, 128 * m:128 * (m + 1)],
                                         g_t[k][:, nlo:nlo + ncnt],
                                         start=(k == 0), stop=(k == NCB - 1))
                    if not rev:
                        nc.vector.tensor_copy(out=ym[m][:, nlo:nlo + ncnt],
                                              in_=ps[:, 0:ncnt])
                    else:
                        nc.vector.tensor_tensor(
                            out=ym[m][:, LM - nlo - ncnt:LM - nlo],
                            in0=ym[m][:, LM - nlo - ncnt:LM - nlo],
                            in1=_rev_free(ps[:, 0:ncnt]), op=OP.add)

    # ---------------- residual 1 + LN2 ----------------
    x2_pool = ctx.enter_context(tc.tile_pool(name="x2_pool", bufs=1))
    x2T = [x2_pool.tile([128, LM], F32, name=f"x2_{m}", tag=f"x2_{m}")
           for m in range(NMT_DM)]
    wk2 = ctx.enter_context(tc.tile_pool(name="wk2", bufs=2))
    xT_r = io["x_mainT"].rearrange("(n p) t -> n p t", p=128)
    x2h, x2sq = [], []
    for m in range(NMT_DM):
        xt = wk2.tile([128, LM], F32, name="xmt", tag="wtmp", bufs=4)
        nc.gpsimd.dma_start(out=xt[:, :], in_=xT_r[m])
        yk = wk2.tile([128, LM], F32, name="yk", tag="wtmp", bufs=4)
        nc.vector.tensor_tensor(out=yk[:, :], in0=ym[m][:, :], in1=keep_rep,
                                op=OP.mult)
        nc.vector.tensor_tensor(out=x2T[m][:, :], in0=xt[:, :], in1=yk[:, :],
                                op=OP.add)
        x2h_m = x2_pool.tile([128, LM], F16, name=f"x2h{m}", tag="x2aux", bufs=8)
        x2sq_m = x2_pool.tile([128, LM], F16, name=f"x2q{m}", tag="x2aux", bufs=8)
        nc.scalar.activation(out=x2h_m[:, :], in_=x2T[m][:, :], func=AF.Copy)
        nc.scalar.activation(out=x2sq_m[:, :], in_=x2T[m][:, :], func=AF.Square)
        x2h.append(x2h_m); x2sq.append(x2sq_m)

    mrow = wk2.tile([1, LM], F32, name="mrow", tag="mrow", bufs=1)
    vrow = wk2.tile([1, LM], F32, name="vrow", tag="vrow", bufs=1)
    for (dst, src) in ((mrow, x2h), (vrow, x2sq)):
        for (nlo, ncnt) in MCH:
            ps = psum.tile([1, 512], F32, name="ln2ps", tag="mm")
            for m in range(NMT_DM):
                nc.tensor.matmul(ps[0:1, 0:ncnt], ones16,
                                 src[m][:, nlo:nlo + ncnt],
                                 start=(m == 0), stop=(m == NMT_DM - 1))
            nc.vector.tensor_copy(out=dst[:, nlo:nlo + ncnt], in_=ps[0:1, 0:ncnt])
    nc.scalar.mul(out=mrow[:, :], in_=mrow[:, :], mul=1.0 / D_MODEL)
    nc.scalar.mul(out=vrow[:, :], in_=vrow[:, :], mul=1.0 / D_MODEL)
    msq = wk2.tile([1, LM], F32, name="msq", tag="msq", bufs=1)
    nc.vector.tensor_tensor(out=msq[:, :], in0=mrow[:, :], in1=mrow[:, :],
                            op=OP.mult)
    nc.vector.tensor_tensor(out=vrow[:, :], in0=vrow[:, :], in1=msq[:, :],
                            op=OP.subtract)
    nc.scalar.activation(out=vrow[:, :], in_=vrow[:, :], func=AF.Sqrt,
                         bias=eps_t[0:1, 0:1], scale=1.0)
    nc.vector.reciprocal(out=vrow[:, :], in_=vrow[:, :])
    nc.gpsimd.dma_start(out=mr_scr[0:1, :], in_=mrow[:, :])
    nc.gpsimd.dma_start(out=mr_scr[1:2, :], in_=vrow[:, :])
    m_rep = wk2.tile([128, LM], F32, name="m_rep", tag="m_rep", bufs=1)
    r_rep = wk2.tile([128, LM], F32, name="r_rep", tag="r_rep", bufs=1)
    nc.gpsimd.dma_start(out=m_rep[:, :], in_=_bcast_row(mr_scr, 0, LM))
    nc.gpsimd.dma_start(out=r_rep[:, :], in_=_bcast_row(mr_scr, 1, LM))

    x2n = []
    for m in range(NMT_DM):
        df = wk2.tile([128, LM], F32, name="df", tag="wtmp", bufs=4)
        nc.vector.tensor_tensor(out=df[:, :], in0=x2T[m][:, :], in1=m_rep[:, :],
                                op=OP.subtract)
        nc.vector.tensor_tensor(out=df[:, :], in0=df[:, :], in1=r_rep[:, :],
                                op=OP.mult)
        x2n_m = x2_pool.tile([128, LM], F16, name=f"x2n{m}", tag="x2aux", bufs=8)
        nc.vector.tensor_scalar(out=x2n_m[:, :], in0=df[:, :],
                                scalar1=n2g_c(m), scalar2=n2b_c(m),
                                op0=OP.mult, op1=OP.add)
        x2n.append(x2n_m)

    # ---------------- FFN ----------------
    wffn = ctx.enter_context(tc.tile_pool(name="wffn", bufs=1))
    w1_t = wffn.tile([128, NKC_DM, D_FF], F16, name="w1_t", tag="w1_t")
    nc.gpsimd.dma_start(out=w1_t[:, :, :],
                      in_=io["ffn_w1"].rearrange("(n p) c -> p n c", p=128))
    gact = []
    for mt in range(NMT_FF):
        ga = wffn.tile([128, LM], F16, name=f"ga{mt}", tag=f"ga{mt}")
        for (nlo, ncnt) in MCH:
            ps = psum.tile([128, 512], F32, name="mm", tag="mm")
            for k in range(NKC_DM):
                nc.tensor.matmul(ps[:, 0:ncnt],
                                 w1_t[:, k, 128 * mt:128 * (mt + 1)],
                                 x2n[k][:, nlo:nlo + ncnt],
                                 start=(k == 0), stop=(k == NKC_DM - 1))
            u = wk2.tile([128, 512], F16, name="gu", tag="gu", bufs=3)
            nc.scalar.activation(out=u[:, 0:ncnt], in_=ps[:, 0:ncnt],
                                 func=AF.Identity, bias=fb1_c(mt),
                                 scale=1.0)
            sq = wk2.tile([128, 512], F16, name="gsq", tag="gsq", bufs=3)
            nc.scalar.activation(out=sq[:, 0:ncnt], in_=u[:, 0:ncnt],
                                 func=AF.Square)
            v = wk2.tile([128, 512], F16, name="gv", tag="gv", bufs=3)
            nc.vector.tensor_scalar(out=v[:, 0:ncnt], in0=sq[:, 0:ncnt],
                                    scalar1=0.044715, scalar2=1.0,
                                    op0=OP.mult, op1=OP.add)
            nc.vector.tensor_tensor(out=v[:, 0:ncnt], in0=u[:, 0:ncnt],
                                    in1=v[:, 0:ncnt], op=OP.mult)
            nc.scalar.activation(out=v[:, 0:ncnt], in_=v[:, 0:ncnt],
                                 func=AF.Tanh, scale=0.7978845608028654)
            nc.vector.tensor_scalar(out=v[:, 0:ncnt], in0=v[:, 0:ncnt],
                                    scalar1=0.5, scalar2=0.5,
                                    op0=OP.mult, op1=OP.add)
            nc.vector.tensor_tensor(out=ga[:, nlo:nlo + ncnt], in0=u[:, 0:ncnt],
                                    in1=v[:, 0:ncnt], op=OP.mult)
        gact.append(ga)
    w2_t = wffn.tile([128, NMT_FF, D_MODEL], F16, name="w2_t", tag="w2_t")
    nc.gpsimd.dma_start(out=w2_t[:, :, :],
                      in_=io["ffn_w2"].rearrange("(n p) c -> p n c", p=128))
    for m in range(NMT_DM):
        for (nlo, ncnt) in MCH:
            ps = psum.tile([128, 512], F32, name="mm", tag="mm")
            for k in range(NMT_FF):
                nc.tensor.matmul(ps[:, 0:ncnt],
                                 w2_t[:, k, 128 * m:128 * (m + 1)],
                                 gact[k][:, nlo:nlo + ncnt],
                                 start=(k == 0), stop=(k == NMT_FF - 1))
            ot = wk2.tile([128, 512], F32, name="ot", tag="ot", bufs=3)
            nc.vector.scalar_tensor_tensor(
                out=ot[:, 0:ncnt], in0=ps[:, 0:ncnt],
                scalar=fb2_c(m), in1=x2T[m][:, nlo:nlo + ncnt],
                op0=OP.add, op1=OP.add)
            nc.gpsimd.dma_start(out=outT[128 * m:128 * (m + 1), nlo:nlo + ncnt],
                              in_=ot[:, 0:ncnt])


_PROGRAM = None


def _get_program():
    global _PROGRAM
    if _PROGRAM is None:
        _PROGRAM = build_program()
    return _PROGRAM


def _prep_inputs(inputs):
    f32 = lambda a: np.ascontiguousarray(np.asarray(a, dtype=np.float32))
    f16 = lambda a: np.ascontiguousarray(
        np.asarray(a, dtype=np.float32).astype(np.float16))
    x = f32(inputs["x"])
    mask = np.asarray(inputs["padding_mask"]).astype(np.float32)

    # consolidated fp32 const block (shared part)
    c32 = np.zeros((128, W32), np.float32)
    c32[:, 0] = EPS
    c32[:, 10:14] = f32(inputs["norm2_g"]).reshape(NMT_DM, 128).T
    c32[:, 14:18] = f32(inputs["norm2_b"]).reshape(NMT_DM, 128).T
    c32[:, 18:22] = f32(inputs["ffn_b2"]).reshape(NMT_DM, 128).T
    c32[:, 22:38] = f32(inputs["ffn_b1"]).reshape(NMT_FF, 128).T
    for di, d in enumerate(("f", "b")):
        base = 38 + di * 184
        cw = f32(inputs["convw_" + d]).reshape(NCB, 128, D_CONV)
        c32[:, base:base + 32] = cw.transpose(1, 0, 2).reshape(128, 32)
        c32[:, base + 32:base + 40] = f32(inputs["convb_" + d]).reshape(NCB, 128).T
        c32[:, base + 40:base + 48] = f32(inputs["dtb_" + d]).reshape(NCB, 128).T
        c32[:, base + 48:base + 56] = f32(inputs["dparam_" + d]).reshape(NCB, 128).T
        A = -np.exp(np.asarray(inputs["alog_" + d], dtype=np.float64)).astype(
            np.float32)
        c32[:, base + 56:base + 184] = A.reshape(NCB, 128, D_STATE).transpose(
            1, 0, 2).reshape(128, 128)

    c16s = np.zeros((128, W16), np.float16)
    c16s[:, 0:128] = np.eye(128, dtype=np.float16)
    c16s[:, 128] = 1.0
    c16s[:, 129:129 + D_MODEL] = f16(inputs["norm1_g"]).reshape(1, D_MODEL)
    c16s[:, 129 + D_MODEL:129 + 2 * D_MODEL] = f16(inputs["norm1_b"]).reshape(
        1, D_MODEL)

    shared = {
        "cst32": c32,
        "ffn_w1": f16(inputs["ffn_w1"]),
        "ffn_w2": f16(inputs["ffn_w2"]),
    }
    for d in ("f", "b"):
        shared["inw_" + d] = f16(inputs["inw_" + d])
        shared["xpw_" + d] = f16(inputs["xpw_" + d])
        shared["dtw_" + d] = f16(inputs["dtw_" + d])
        shared["outw_" + d] = f16(inputs["outw_" + d])

    in_maps = []
    for core in range(8):
        b, half = divmod(core, 2)
        s = half * LM
        lo, hi = max(0, s - H), min(L, s + LM + H)
        x_ext = np.zeros((LEXT, D_MODEL), np.float32)
        keep_ext = np.zeros((LEXT,), np.float32)
        x_ext[lo - (s - H):hi - (s - H)] = x[b, lo:hi]
        keep_ext[lo - (s - H):hi - (s - H)] = 1.0 - mask[b, lo:hi]
        m = dict(shared)
        m["x_ext"] = x_ext
        m["x_mainT"] = np.ascontiguousarray(x[b, s:s + LM].T)
        cc = c32.copy()
        cc[:, 1:1 + LEXT // 128] = keep_ext.reshape(LEXT // 128, 128).T
        m["cst32"] = cc
        c16c = c16s.copy()
        c16c[:, 129 + 2 * D_MODEL:] = keep_ext[H:H + LM].astype(np.float16)[None, :]
        m["cst16"] = c16c
        in_maps.append(m)
    return in_maps


def kernel(**inputs):
    from concourse.bass_utils import run_bass_kernel_spmd
    nc = _get_program()
    in_maps = _prep_inputs(inputs)
    res = run_bass_kernel_spmd(nc, in_maps, core_ids=list(range(8)))
    out = np.zeros((B, L, D_MODEL), np.float32)
    for core in range(8):
        b, half = divmod(core, 2)
        out[b, half * LM:(half + 1) * LM] = res.results[core]["outT"].T
    return out


# revision 16
# speedup vs baseline: 1.1185x; 1.1185x over previous
"""Bidirectional Mamba encoder layer on 8 Trainium2 NeuronCores.

Sharding: 8-way data parallel over (batch=4) x (seq halves=2). Each core
processes one (b, half) slice with a HALO-token warm-up window on each side
so the selective scans start from a decayed (effectively exact) state.
No collectives needed.

Per-core dataflow (channel-major):
  LN1 (token-major) -> transpose -> h_T
  per direction d in (fwd, bwd):          # bwd handled by storing reversed time
    inproj matmul -> xin_T (ext, +3 zero pad), z (silu, main only)
    depthwise conv (DVE shifts) + silu -> xc_T
    xp-proj -> dt_r, B, C;  dt-proj -> softplus (Exp+Ln) -> dt; dtx = dt*xc
    scan: for n in 16: broadcast B_n/C_n rows to 128 partitions (DMA);
          for cb in 8: dA = Exp(A[:,n]*dt) (ACT, per-partition scale);
                       dBx = dtx * B_rep; h = tensor_tensor_scan(dA, dBx);
                       y_acc += h * C_rep (main window only)
    gate: (y_acc + xc*dparam) * silu(z); outproj matmul (f stored, b added rev)
  residual + LN2 (channel-major, mean/var via ones-matmul) + FFN (gelu tanh)
Output: out^T [512, 1024] per core; host reassembles.
"""
import os
import sys
import numpy as np
from contextlib import ExitStack

for _p in ("/opt/trn_rl_repo", "/opt/pypackages"):
    if _p not in sys.path and os.path.isdir(_p):
        sys.path.append(_p)

import concourse.bass as bass
import concourse.mybir as mybir
import concourse.tile as tile

F32 = mybir.dt.float32
F16 = mybir.dt.float16
AF = mybir.ActivationFunctionType
OP = mybir.AluOpType


# ---------------------------------------------------------------------------
# This walrus build accepts at most ONE sync-wait command per instruction.
# Tile emits several; hoist all but one onto same-engine NoOps (engines
# dispatch in program order, so this is semantically identical).
import json as _json


def _split_waits(bir_json_bytes, max_waits=1):
    d = _json.loads(bir_json_bytes)
    uid = [0]
    for fn in d.get("functions", []):
        for blk in fn.get("blocks", []):
            ins_list = blk.get("instructions")
            if not ins_list:
                continue
            out = []
            for ins in ins_list:
                si = ins.get("sync_info") or {}
                waits = si.get("on_wait") or []
                if len(waits) > max_waits:
                    keep = waits[-max_waits:]
                    for w in waits[:-max_waits]:
                        uid[0] += 1
                        out.append({
                            "name": f"{ins['name']}-sw{uid[0]}",
                            "opcode": "NoOp",
                            "engine": ins["engine"],
                            "ins": [],
                            "outs": [],
                            "sync_info": {"on_wait": [w]},
                        })
                    si["on_wait"] = keep
                    ins["sync_info"] = si
                out.append(ins)
            blk["instructions"] = out
    return _json.dumps(d).encode()


def _install_split_waits():
    if getattr(mybir, "_ant_split_waits_installed", False):
        return
    _orig = mybir.module_to_json_bytes

    def _patched(m, *a, **k):
        return _split_waits(_orig(m, *a, **k))

    mybir.module_to_json_bytes = _patched
    mybir._ant_split_waits_installed = True


_install_split_waits()

D_MODEL = 512
D_INNER = 1024
D_STATE = 16
D_CONV = 4
DT_RANK = 32
D_FF = 2048
B, L = 4, 2048
EPS = 1e-5

H = 64                 # halo tokens on each side of the main window
LM = 1024              # main tokens per core
LEXT = LM + 2 * H      # 1152
NCB = D_INNER // 128   # 8 channel blocks
NMT_DM = D_MODEL // 128  # 4 d_model tiles
NMT_FF = D_FF // 128     # 16 ff tiles
NKC_DM = D_MODEL // 128  # 4 k-chunks for d_model contraction

MAIN_LO, MAIN_HI = H, H + LM

SL = MAIN_HI               # per-direction window length (1088)
W32 = 38 + 2 * 184          # consolidated fp32 consts width
W16 = 129 + 2 * D_MODEL + LM  # ident | ones | n1g | n1b | keep_rep


def _chunks(total, size=512):
    return [(i, min(size, total - i)) for i in range(0, total, size)]

WCH = _chunks(SL)       # chunks for direction-window matmuls
MCH = _chunks(LM)       # time chunks for main-width matmuls


def _rev_free(ap):
    """Reverse the innermost (free) axis of a 2D AP view."""
    (pstep, pcnt), (fstep, fcnt) = ap.ap[0], ap.ap[1]
    return bass.AP(
        tensor=ap.tensor,
        offset=ap.offset + (fcnt - 1) * fstep,
        ap=[[pstep, pcnt], [-fstep, fcnt]],
    )


def _bcast_rows(dram, row0, nrows, width, nparts=128):
    """AP reading `nrows` DRAM rows, each broadcast across nparts partitions."""
    base = dram[row0:row0 + nrows, 0:width]
    return bass.AP(
        tensor=base.tensor,
        offset=base.offset,
        ap=[[0, nparts], [width, nrows], [1, width]],
    )


def _bcast_row(dram, row, width, nparts=128):
    """AP reading one DRAM row broadcast across nparts partitions."""
    base = dram[row:row + 1, 0:width]
    return bass.AP(
        tensor=base.tensor,
        offset=base.offset,
        ap=[[0, nparts], [1, width]],
    )


def build_program():
    nc = bass.Bass("TRN2", target_bir_lowering=False, debug=False)

    io = {}
    def din(name, shape, dt=F32):
        io[name] = nc.dram_tensor(name, shape, dt, kind="ExternalInput").ap()
        return io[name]

    din("x_ext", [LEXT, D_MODEL])
    din("x_mainT", [D_MODEL, LM])
    din("cst32", [128, W32])
    din("cst16", [128, W16], F16)
    for d in ("f", "b"):
        din("inw_" + d, [D_MODEL, 2 * D_INNER], F16)
        din("xpw_" + d, [D_INNER, DT_RANK + 2 * D_STATE], F16)
        din("dtw_" + d, [DT_RANK, D_INNER], F16)
        din("outw_" + d, [D_INNER, D_MODEL], F16)
    din("ffn_w1", [D_MODEL, D_FF], F16)
    din("ffn_w2", [D_FF, D_MODEL], F16)

    outT = nc.dram_tensor("outT", [D_MODEL, LM], F32, kind="ExternalOutput").ap()
    bc_scr = {d: nc.dram_tensor(f"bc_scr_{d}", [2 * D_STATE, SL], F16,
                                kind="Internal").ap() for d in ("f", "b")}
    mr_scr = nc.dram_tensor("mr_scr", [2, LM], F32, kind="Internal").ap()

    with tile.TileContext(nc) as tc, ExitStack() as ctx:
        _emit(ctx, tc, nc, io, outT, bc_scr, mr_scr)
    return nc


def _emit(ctx, tc, nc, io, outT, bc_scr, mr_scr):
    psum = ctx.enter_context(tc.tile_pool(name="psum", bufs=4, space="PSUM"))
    psum_tp = ctx.enter_context(tc.tile_pool(name="psum_tp", bufs=2, space="PSUM"))
    const = ctx.enter_context(tc.tile_pool(name="const", bufs=1))

    # ---------------- constants: two consolidated blocks ----------------
    c32 = const.tile([128, W32], F32, name="c32", tag="c32")
    nc.gpsimd.dma_start(out=c32[:, :], in_=io["cst32"][:, :])
    c16 = const.tile([128, W16], F16, name="c16", tag="c16")
    nc.gpsimd.dma_start(out=c16[:, :], in_=io["cst16"][:, :])

    eps_t = c32[:, 0:1]
    def keep_tok(i):
        return c32[:, 1 + i:2 + i]
    def n2g_c(m):
        return c32[:, 10 + m:11 + m]
    def n2b_c(m):
        return c32[:, 14 + m:15 + m]
    def fb2_c(m):
        return c32[:, 18 + m:19 + m]
    def fb1_c(mt):
        return c32[:, 22 + mt:23 + mt]
    def _db(d):
        return 38 + (0 if d == "f" else 184)
    def convw_c(d, c, k):
        o = _db(d) + c * 4 + k
        return c32[:, o:o + 1]
    def convb_c(d, c):
        o = _db(d) + 32 + c
        return c32[:, o:o + 1]
    def dtb_c(d, c):
        o = _db(d) + 40 + c
        return c32[:, o:o + 1]
    def dparam_c(d, c):
        o = _db(d) + 48 + c
        return c32[:, o:o + 1]
    def A_c(d, c, n):
        o = _db(d) + 56 + c * 16 + n
        return c32[:, o:o + 1]

    ident = c16[:, 0:128]
    ones16 = c16[:, 128:129]
    n1g_rep = c16[:, 129:129 + D_MODEL]
    n1b_rep = c16[:, 129 + D_MODEL:129 + 2 * D_MODEL]
    keep_rep = c16[:, 129 + 2 * D_MODEL:129 + 2 * D_MODEL + LM]

    # ---------------- Phase 1: LN1 + transpose -> h_T ----------------
    hT_pool = ctx.enter_context(tc.tile_pool(name="hT_pool", bufs=1))
    hT = [hT_pool.tile([128, LEXT], F16, name=f"hT{j}", tag=f"hT{j}")
          for j in range(NKC_DM)]

    with tc.tile_pool(name="p1", bufs=3) as p1:
        x_tiled = io["x_ext"].rearrange("(n p) d -> n p d", p=128)
        for i in range(LEXT // 128):
            xt = p1.tile([128, D_MODEL], F32, name="xt", tag="xt", bufs=9)
            nc.gpsimd.dma_start(out=xt[:, :], in_=x_tiled[i])
            stats = p1.tile([128, 6], F32, name="stats", tag="stats")
            nc.vector.bn_stats(out=stats[:, :], in_=xt[:, :])
            mv = p1.tile([128, 2], F32, name="mv", tag="mv")
            nc.vector.bn_aggr(out=mv[:, :], in_=stats[:, :])
            nc.scalar.activation(out=mv[:, 1:2], in_=mv[:, 1:2], func=AF.Sqrt,
                                 bias=eps_t, scale=1.0)
            nc.vector.reciprocal(out=mv[:, 1:2], in_=mv[:, 1:2])
            xn = p1.tile([128, D_MODEL], F32, name="xn", tag="xn")
            nc.vector.tensor_scalar(out=xn[:, :], in0=xt[:, :],
                                    scalar1=mv[:, 0:1], scalar2=mv[:, 1:2],
                                    op0=OP.subtract, op1=OP.mult)
            tmp = p1.tile([128, D_MODEL], F32, name="tmp", tag="tmp")
            nc.vector.scalar_tensor_tensor(out=tmp[:, :], in0=xn[:, :],
                                           scalar=keep_tok(i),
                                           in1=n1g_rep[:, :],
                                           op0=OP.mult, op1=OP.mult)
            htok = p1.tile([128, D_MODEL], F16, name="htok", tag="htok")
            nc.vector.scalar_tensor_tensor(out=htok[:, :], in0=n1b_rep[:, :],
                                           scalar=keep_tok(i), in1=tmp[:, :],
                                           op0=OP.mult, op1=OP.add)
            for j in range(NKC_DM):
                ps = psum_tp.tile([128, 128], F16, name="tp", tag="tp")
                nc.tensor.transpose(ps[:, :], htok[:, 128 * j:128 * (j + 1)],
                                    ident)
                nc.vector.tensor_copy(out=hT[j][:, 128 * i:128 * (i + 1)],
                                      in_=ps[:, :])

    # ---------------- per-direction pipeline ----------------
    # Each direction works in "window" coordinates w in [0, SL): for fwd
    # w = tau in [0, 1088); for bwd w = LEXT-1-tau (tau in [64, 1152)), i.e.
    # time-reversed. The last HALO tokens of the ext window are only state
    # warm-up for the opposite direction and are skipped entirely.
    ym_pool = ctx.enter_context(tc.tile_pool(name="ym_pool", bufs=1))
    ym = [ym_pool.tile([128, LM], F16, name=f"ym{m}", tag=f"ym{m}")
          for m in range(NMT_DM)]
    wpool = ctx.enter_context(tc.tile_pool(name="wpool", bufs=1))

    for d in ("f", "b"):
        rev = (d == "b")
        # inproj tau-chunks for this direction's window
        ICH = [(0, 512), (512, 512), (1024, 64)] if not rev else \
              [(64, 512), (576, 512), (1088, 64)]
        with tc.tile_pool(name="dirp" + d, bufs=1) as dp, \
             tc.tile_pool(name="work" + d, bufs=2) as wk:
            inw_t = [wpool.tile([128, 2 * D_INNER], F16, name=f"inw{k}",
                                tag=f"inw{k}") for k in range(NKC_DM)]
            inw_r = io["inw_" + d].rearrange("(n p) c -> n p c", p=128)
            for k in range(NKC_DM):
                nc.gpsimd.dma_start(out=inw_t[k][:, :], in_=inw_r[k])

            # big tag group: xin / xc / dt / dtx share recycled slots
            def big(nm):
                return dp.tile([128, 3 + SL], F16, name=nm, tag="big", bufs=17)

            xin = [big(f"xin{c}") for c in range(NCB)]
            z_s = [dp.tile([128, LM], F16, name=f"z{c}", tag="z", bufs=NCB)
                   for c in range(NCB)]
            for c in range(NCB):
                nc.vector.memset(xin[c][:, 0:3], 0.0)

            # ---- inproj ----
            for mt in range(2 * NCB):
                for (nlo, ncnt) in ICH:
                    ps = psum.tile([128, 512], F32, name="mm", tag="mm")
                    for k in range(NKC_DM):
                        nc.tensor.matmul(ps[:, 0:ncnt],
                                         inw_t[k][:, 128 * mt:128 * (mt + 1)],
                                         hT[k][:, nlo:nlo + ncnt],
                                         start=(k == 0), stop=(k == NKC_DM - 1))
                    if mt < NCB:  # xin part -> window coords, ACT evac
                        if not rev:
                            nc.scalar.copy(
                                out=xin[mt][:, 3 + nlo:3 + nlo + ncnt],
                                in_=ps[:, 0:ncnt])
                        else:
                            w0 = LEXT - (nlo + ncnt)
                            nc.scalar.copy(
                                out=xin[mt][:, 3 + w0:3 + w0 + ncnt],
                                in_=_rev_free(ps[:, 0:ncnt]))
                    else:       # z part -> silu, main window only
                        c = mt - NCB
                        lo, hi = max(MAIN_LO, nlo), min(MAIN_HI, nlo + ncnt)
                        if lo >= hi:
                            continue
                        zr = wk.tile([128, 512], F16, name="zraw", tag="zraw")
                        zg = wk.tile([128, 512], F16, name="zsig", tag="zsig")
                        if not rev:
                            nc.scalar.copy(out=zr[:, 0:hi - lo],
                                           in_=ps[:, lo - nlo:hi - nlo])
                        else:
                            nc.scalar.copy(
                                out=zr[:, 0:hi - lo],
                                in_=_rev_free(ps[:, lo - nlo:hi - nlo]))
                        nc.scalar.activation(out=zg[:, 0:hi - lo],
                                             in_=zr[:, 0:hi - lo], func=AF.Sigmoid)
                        o0 = (lo - MAIN_LO) if not rev else (MAIN_HI - hi)
                        nc.vector.tensor_tensor(out=z_s[c][:, o0:o0 + hi - lo],
                                                in0=zr[:, 0:hi - lo],
                                                in1=zg[:, 0:hi - lo], op=OP.mult)

            # ---- conv (taps via tensor_scalar + DMA accumulate) + silu ----
            xc = [big(f"xc{c}") for c in range(NCB)]
            for c in range(NCB):
                acc = wk.tile([128, SL], F16, name="cacc", tag="cacc")
                nc.vector.tensor_scalar(out=acc[:, :], in0=xin[c][:, 0:SL],
                                        scalar1=convw_c(d, c, 0),
                                        scalar2=convb_c(d, c),
                                        op0=OP.mult, op1=OP.add)
                for k in range(1, D_CONV):
                    tap = wk.tile([128, SL], F16, name="ctap", tag="ctap", bufs=3)
                    nc.vector.tensor_scalar(out=tap[:, :],
                                            in0=xin[c][:, k:k + SL],
                                            scalar1=convw_c(d, c, k),
                                            scalar2=None, op0=OP.mult)
                    nc.gpsimd.dma_start(out=acc[:, :], in_=tap[:, :],
                                        accum_op=OP.add)
                sg = wk.tile([128, SL], F16, name="csig", tag="csig")
                nc.scalar.activation(out=sg[:, :], in_=acc[:, :], func=AF.Sigmoid)
                nc.vector.tensor_tensor(out=xc[c][:, 0:SL], in0=acc[:, :],
                                        in1=sg[:, :], op=OP.mult)

            # ---- xp proj -> dt_r, B, C ----
            xpw_t = wpool.tile([128, NCB, 2 * D_STATE + DT_RANK], F16,
                               name="xpw_t", tag="xpw_t")
            nc.gpsimd.dma_start(out=xpw_t[:, :, :],
                              in_=io["xpw_" + d].rearrange("(n p) w -> p n w",
                                                           p=128))
            dtr = dp.tile([DT_RANK, SL], F16, name="dtr", tag="dtr")
            bc_sb = wk.tile([2 * D_STATE, SL], F16, name="bc_sb", tag="bc_sb")
            for (nlo, ncnt) in WCH:
                ps = psum.tile([64, 512], F32, name="xp", tag="mm")
                for k in range(NCB):
                    nc.tensor.matmul(ps[:, 0:ncnt], xpw_t[:, k, :],
                                     xc[k][:, nlo:nlo + ncnt],
                                     start=(k == 0), stop=(k == NCB - 1))
                nc.scalar.copy(out=dtr[:, nlo:nlo + ncnt],
                               in_=ps[0:DT_RANK, 0:ncnt])
                nc.scalar.copy(out=bc_sb[:, nlo:nlo + ncnt],
                               in_=ps[DT_RANK:DT_RANK + 2 * D_STATE, 0:ncnt])
            nc.gpsimd.dma_start(out=bc_scr[d][:, :], in_=bc_sb[:, :])

            # ---- dt proj + softplus + dtx + xc main copy ----
            dtw_t = wpool.tile([DT_RANK, D_INNER], F16, name="dtw_t", tag="dtw_t")
            nc.gpsimd.dma_start(out=dtw_t[:, :], in_=io["dtw_" + d][:, :])
            dt_t, dtx, xcm = [], [], []
            for c in range(NCB):
                dt_c = big(f"dt{c}")
                for (nlo, ncnt) in WCH:
                    ps = psum.tile([128, 512], F32, name="mm", tag="mm")
                    nc.tensor.matmul(ps[:, 0:ncnt],
                                     dtw_t[:, 128 * c:128 * (c + 1)],
                                     dtr[:, nlo:nlo + ncnt], start=True, stop=True)
                    # softplus(x) = ln(1 + exp(x)); same ACT set as scan's Exp
                    nc.scalar.activation(out=ps[:, 0:ncnt], in_=ps[:, 0:ncnt],
                                         func=AF.Exp,
                                         bias=dtb_c(d, c), scale=1.0)
                    nc.scalar.activation(out=dt_c[:, nlo:nlo + ncnt],
                                         in_=ps[:, 0:ncnt], func=AF.Ln,
                                         bias=1.0, scale=1.0)
                dtx_c = big(f"dtx{c}")
                nc.vector.tensor_tensor(out=dtx_c[:, 0:SL], in0=dt_c[:, 0:SL],
                                        in1=xc[c][:, 0:SL], op=OP.mult)
                xcm_c = dp.tile([128, LM], F16, name=f"xcm{c}", tag="xcm",
                                bufs=NCB)
                nc.vector.tensor_copy(out=xcm_c[:, :],
                                      in_=xc[c][:, MAIN_LO:MAIN_HI])
                dt_t.append(dt_c); dtx.append(dtx_c); xcm.append(xcm_c)

            # ---- scan over states (y accumulated via DMA accumulate) ----
            yacc = [dp.tile([128, LM], F16, name=f"ya{c}", tag="yg",
                            bufs=NCB + 1) for c in range(NCB)]
            with tc.tile_pool(name="scan" + d, bufs=2) as sp, \
                 tc.tile_pool(name="bcrep" + d, bufs=2) as bp:
                for g in range(D_STATE // 2):
                    b_grp = bp.tile([128, 2, SL], F16, name="b_grp", tag="b_grp")
                    c_grp = bp.tile([128, 2, SL], F16, name="c_grp", tag="c_grp")
                    nc.gpsimd.dma_start(out=b_grp[:, :, :],
                                        in_=_bcast_rows(bc_scr[d], 2 * g, 2, SL))
                    nc.gpsimd.dma_start(
                        out=c_grp[:, :, :],
                        in_=_bcast_rows(bc_scr[d], D_STATE + 2 * g, 2, SL))
                    for nn in range(2):
                        n = 2 * g + nn
                        b_rep = b_grp[:, nn, :]
                        c_rep = c_grp[:, nn, :]
                        for c in range(NCB):
                            dA = sp.tile([128, SL], F16, name="dA", tag="dA")
                            nc.scalar.activation(out=dA[:, :],
                                                 in_=dt_t[c][:, 0:SL],
                                                 func=AF.Exp,
                                                 scale=A_c(d, c, n))
                            dBx = sp.tile([128, SL], F16, name="dBx", tag="dBx")
                            nc.vector.tensor_tensor(out=dBx[:, :],
                                                    in0=dtx[c][:, 0:SL],
                                                    in1=b_rep, op=OP.mult)
                            hsc = sp.tile([128, SL], F16, name="hsc", tag="hsc")
                            nc.vector.tensor_tensor_scan(
                                out=hsc[:, :], data0=dA[:, :], data1=dBx[:, :],
                                initial=0.0, op0=OP.mult, op1=OP.add)
                            if n == 0:
                                nc.vector.tensor_tensor(
                                    out=yacc[c][:, :],
                                    in0=hsc[:, MAIN_LO:SL],
                                    in1=c_rep[:, MAIN_LO:SL], op=OP.mult)
                            else:
                                hc = sp.tile([128, LM], F16, name="hc", tag="hc",
                                             bufs=3)
                                nc.vector.tensor_tensor(
                                    out=hc[:, :], in0=hsc[:, MAIN_LO:SL],
                                    in1=c_rep[:, MAIN_LO:SL], op=OP.mult)
                                nc.gpsimd.dma_start(out=yacc[c][:, :],
                                                    in_=hc[:, :],
                                                    accum_op=OP.add)

            # ---- gate ----
            g_t = []
            for c in range(NCB):
                t1 = wk.tile([128, LM], F16, name="gate1", tag="gate1")
                nc.vector.tensor_scalar(out=t1[:, :], in0=xcm[c][:, :],
                                        scalar1=dparam_c(d, c), scalar2=None,
                                        op0=OP.mult)
                t2 = wk.tile([128, LM], F16, name="gate2", tag="gate2")
                nc.vector.tensor_tensor(out=t2[:, :], in0=t1[:, :],
                                        in1=yacc[c][:, :], op=OP.add)
                g_c = dp.tile([128, LM], F16, name=f"g{c}", tag="yg",
                              bufs=NCB + 1)
                nc.vector.tensor_tensor(out=g_c[:, :], in0=t2[:, :],
                                        in1=z_s[c][:, :], op=OP.mult)
                g_t.append(g_c)

            # ---- outproj (f: copy, b: add reversed) ----
            outw_t = wpool.tile([128, NCB, D_MODEL], F16, name="outw_t",
                                tag="outw_t")
            nc.gpsimd.dma_start(out=outw_t[:, :, :],
                              in_=io["outw_" + d].rearrange("(n p) w -> p n w",
                                                            p=128))
            for m in range(NMT_DM):
                for (nlo, ncnt) in MCH:
                    ps = psum.tile([128, 512], F32, name="mm", tag="mm")
                    for k in range(NCB):
                        nc.tensor.matmul(ps[:, 0:ncnt],
                                         outw_t[:, k, 128 * m:128 * (m + 1)],
                                         g_t[k][:, nlo:nlo + ncnt],
                                         start=(k == 0), stop=(k == NCB - 1))
                    if not rev:
                        nc.scalar.copy(out=ym[m][:, nlo:nlo + ncnt],
                                       in_=ps[:, 0:ncnt])
                    else:
                        nc.vector.tensor_tensor(
                            out=ym[m][:, LM - nlo - ncnt:LM - nlo],
                            in0=ym[m][:, LM - nlo - ncnt:LM - nlo],
                            in1=_rev_free(ps[:, 0:ncnt]), op=OP.add)

    # ---------------- residual 1 + LN2 ----------------
    x2_pool = ctx.enter_context(tc.tile_pool(name="x2_pool", bufs=1))
    x2T = [x2_pool.tile([128, LM], F32, name=f"x2_{m}", tag=f"x2_{m}")
           for m in range(NMT_DM)]
    wk2 = ctx.enter_context(tc.tile_pool(name="wk2", bufs=2))
    xT_r = io["x_mainT"].rearrange("(n p) t -> n p t", p=128)
    x2h, x2sq = [], []
    for m in range(NMT_DM):
        xt = wk2.tile([128, LM], F32, name="xmt", tag="wtmp", bufs=4)
        nc.gpsimd.dma_start(out=xt[:, :], in_=xT_r[m])
        yk = wk2.tile([128, LM], F32, name="yk", tag="wtmp", bufs=4)
        nc.vector.tensor_tensor(out=yk[:, :], in0=ym[m][:, :], in1=keep_rep,
                                op=OP.mult)
        nc.vector.tensor_tensor(out=x2T[m][:, :], in0=xt[:, :], in1=yk[:, :],
                                op=OP.add)
        x2h_m = x2_pool.tile([128, LM], F16, name=f"x2h{m}", tag="x2aux", bufs=8)
        x2sq_m = x2_pool.tile([128, LM], F16, name=f"x2q{m}", tag="x2aux", bufs=8)
        nc.scalar.activation(out=x2h_m[:, :], in_=x2T[m][:, :], func=AF.Copy)
        nc.scalar.activation(out=x2sq_m[:, :], in_=x2T[m][:, :], func=AF.Square)
        x2h.append(x2h_m); x2sq.append(x2sq_m)

    mrow = wk2.tile([1, LM], F32, name="mrow", tag="mrow", bufs=1)
    vrow = wk2.tile([1, LM], F32, name="vrow", tag="vrow", bufs=1)
    for (dst, src) in ((mrow, x2h), (vrow, x2sq)):
        for (nlo, ncnt) in MCH:
            ps = psum.tile([1, 512], F32, name="ln2ps", tag="mm")
            for m in range(NMT_DM):
                nc.tensor.matmul(ps[0:1, 0:ncnt], ones16,
                                 src[m][:, nlo:nlo + ncnt],
                                 start=(m == 0), stop=(m == NMT_DM - 1))
            nc.vector.tensor_copy(out=dst[:, nlo:nlo + ncnt], in_=ps[0:1, 0:ncnt])
    nc.scalar.mul(out=mrow[:, :], in_=mrow[:, :], mul=1.0 / D_MODEL)
    nc.scalar.mul(out=vrow[:, :], in_=vrow[:, :], mul=1.0 / D_MODEL)
    msq = wk2.tile([1, LM], F32, name="msq", tag="msq", bufs=1)
    nc.vector.tensor_tensor(out=msq[:, :], in0=mrow[:, :], in1=mrow[:, :],
                            op=OP.mult)
    nc.vector.tensor_tensor(out=vrow[:, :], in0=vrow[:, :], in1=msq[:, :],
                            op=OP.subtract)
    nc.scalar.activation(out=vrow[:, :], in_=vrow[:, :], func=AF.Sqrt,
                         bias=eps_t[0:1, 0:1], scale=1.0)
    nc.vector.reciprocal(out=vrow[:, :], in_=vrow[:, :])
    nc.gpsimd.dma_start(out=mr_scr[0:1, :], in_=mrow[:, :])
    nc.gpsimd.dma_start(out=mr_scr[1:2, :], in_=vrow[:, :])
    m_rep = wk2.tile([128, LM], F32, name="m_rep", tag="m_rep", bufs=1)
    r_rep = wk2.tile([128, LM], F32, name="r_rep", tag="r_rep", bufs=1)
    nc.gpsimd.dma_start(out=m_rep[:, :], in_=_bcast_row(mr_scr, 0, LM))
    nc.gpsimd.dma_start(out=r_rep[:, :], in_=_bcast_row(mr_scr, 1, LM))

    x2n = []
    for m in range(NMT_DM):
        df = wk2.tile([128, LM], F32, name="df", tag="wtmp", bufs=4)
        nc.vector.tensor_tensor(out=df[:, :], in0=x2T[m][:, :], in1=m_rep[:, :],
                                op=OP.subtract)
        nc.vector.tensor_tensor(out=df[:, :], in0=df[:, :], in1=r_rep[:, :],
                                op=OP.mult)
        x2n_m = x2_pool.tile([128, LM], F16, name=f"x2n{m}", tag="x2aux", bufs=8)
        nc.vector.tensor_scalar(out=x2n_m[:, :], in0=df[:, :],
                                scalar1=n2g_c(m), scalar2=n2b_c(m),
                                op0=OP.mult, op1=OP.add)
        x2n.append(x2n_m)

    # ---------------- FFN ----------------
    wffn = ctx.enter_context(tc.tile_pool(name="wffn", bufs=1))
    w1_t = wffn.tile([128, NKC_DM, D_FF], F16, name="w1_t", tag="w1_t")
    nc.gpsimd.dma_start(out=w1_t[:, :, :],
                      in_=io["ffn_w1"].rearrange("(n p) c -> p n c", p=128))
    gact = []
    for mt in range(NMT_FF):
        ga = wffn.tile([128, LM], F16, name=f"ga{mt}", tag=f"ga{mt}")
        for (nlo, ncnt) in MCH:
            ps = psum.tile([128, 512], F32, name="mm", tag="mm")
            for k in range(NKC_DM):
                nc.tensor.matmul(ps[:, 0:ncnt],
                                 w1_t[:, k, 128 * mt:128 * (mt + 1)],
                                 x2n[k][:, nlo:nlo + ncnt],
                                 start=(k == 0), stop=(k == NKC_DM - 1))
            u = wk2.tile([128, 512], F16, name="gu", tag="gu", bufs=3)
            nc.scalar.activation(out=u[:, 0:ncnt], in_=ps[:, 0:ncnt],
                                 func=AF.Identity, bias=fb1_c(mt),
                                 scale=1.0)
            sq = wk2.tile([128, 512], F16, name="gsq", tag="gsq", bufs=3)
            nc.scalar.activation(out=sq[:, 0:ncnt], in_=u[:, 0:ncnt],
                                 func=AF.Square)
            v = wk2.tile([128, 512], F16, name="gv", tag="gv", bufs=3)
            nc.vector.tensor_scalar(out=v[:, 0:ncnt], in0=sq[:, 0:ncnt],
                                    scalar1=0.044715, scalar2=1.0,
                                    op0=OP.mult, op1=OP.add)
            nc.vector.tensor_tensor(out=v[:, 0:ncnt], in0=u[:, 0:ncnt],
                                    in1=v[:, 0:ncnt], op=OP.mult)
            nc.scalar.activation(out=v[:, 0:ncnt], in_=v[:, 0:ncnt],
                                 func=AF.Tanh, scale=0.7978845608028654)
            nc.vector.tensor_scalar(out=v[:, 0:ncnt], in0=v[:, 0:ncnt],
                                    scalar1=0.5, scalar2=0.5,
                                    op0=OP.mult, op1=OP.add)
            nc.vector.tensor_tensor(out=ga[:, nlo:nlo + ncnt], in0=u[:, 0:ncnt],
                                    in1=v[:, 0:ncnt], op=OP.mult)
        gact.append(ga)
    w2_t = wffn.tile([128, NMT_FF, D_MODEL], F16, name="w2_t", tag="w2_t")
    nc.gpsimd.dma_start(out=w2_t[:, :, :],
                      in_=io["ffn_w2"].rearrange("(n p) c -> p n c", p=128))
    for m in range(NMT_DM):
        for (nlo, ncnt) in MCH:
            ps = psum.tile([128, 512], F32, name="mm", tag="mm")
            for k in range(NMT_FF):
                nc.tensor.matmul(ps[:, 0:ncnt],
                                 w2_t[:, k, 128 * m:128 * (m + 1)],
                                 gact[k][:, nlo:nlo + ncnt],
                                 start=(k == 0), stop=(k == NMT_FF - 1))
            ot = wk2.tile([128, 512], F32, name="ot", tag="ot", bufs=3)
            nc.vector.scalar_tensor_tensor(
                out=ot[:, 0:ncnt], in0=ps[:, 0:ncnt],
                scalar=fb2_c(m), in1=x2T[m][:, nlo:nlo + ncnt],
                op0=OP.add, op1=OP.add)
            nc.gpsimd.dma_start(out=outT[128 * m:128 * (m + 1), nlo:nlo + ncnt],
                              in_=ot[:, 0:ncnt])


_PROGRAM = None


def _get_program():
    global _PROGRAM
    if _PROGRAM is None:
        _PROGRAM = build_program()
    return _PROGRAM


def _prep_inputs(inputs):
    f32 = lambda a: np.ascontiguousarray(np.asarray(a, dtype=np.float32))
    f16 = lambda a: np.ascontiguousarray(
        np.asarray(a, dtype=np.float32).astype(np.float16))
    x = f32(inputs["x"])
    mask = np.asarray(inputs["padding_mask"]).astype(np.float32)

    # consolidated fp32 const block (shared part)
    c32 = np.zeros((128, W32), np.float32)
    c32[:, 0] = EPS
    c32[:, 10:14] = f32(inputs["norm2_g"]).reshape(NMT_DM, 128).T
    c32[:, 14:18] = f32(inputs["norm2_b"]).reshape(NMT_DM, 128).T
    c32[:, 18:22] = f32(inputs["ffn_b2"]).reshape(NMT_DM, 128).T
    c32[:, 22:38] = f32(inputs["ffn_b1"]).reshape(NMT_FF, 128).T
    for di, d in enumerate(("f", "b")):
        base = 38 + di * 184
        cw = f32(inputs["convw_" + d]).reshape(NCB, 128, D_CONV)
        c32[:, base:base + 32] = cw.transpose(1, 0, 2).reshape(128, 32)
        c32[:, base + 32:base + 40] = f32(inputs["convb_" + d]).reshape(NCB, 128).T
        c32[:, base + 40:base + 48] = f32(inputs["dtb_" + d]).reshape(NCB, 128).T
        c32[:, base + 48:base + 56] = f32(inputs["dparam_" + d]).reshape(NCB, 128).T
        A = -np.exp(np.asarray(inputs["alog_" + d], dtype=np.float64)).astype(
            np.float32)
        c32[:, base + 56:base + 184] = A.reshape(NCB, 128, D_STATE).transpose(
            1, 0, 2).reshape(128, 128)

    c16s = np.zeros((128, W16), np.float16)
    c16s[:, 0:128] = np.eye(128, dtype=np.float16)
    c16s[:, 128] = 1.0
    c16s[:, 129:129 + D_MODEL] = f16(inputs["norm1_g"]).reshape(1, D_MODEL)
    c16s[:, 129 + D_MODEL:129 + 2 * D_MODEL] = f16(inputs["norm1_b"]).reshape(
        1, D_MODEL)

    shared = {
        "cst32": c32,
        "ffn_w1": f16(inputs["ffn_w1"]),
        "ffn_w2": f16(inputs["ffn_w2"]),
    }
    for d in ("f", "b"):
        shared["inw_" + d] = f16(inputs["inw_" + d])
        shared["xpw_" + d] = f16(inputs["xpw_" + d])
        shared["dtw_" + d] = f16(inputs["dtw_" + d])
        shared["outw_" + d] = f16(inputs["outw_" + d])

    in_maps = []
    for core in range(8):
        b, half = divmod(core, 2)
        s = half * LM
        lo, hi = max(0, s - H), min(L, s + LM + H)
        x_ext = np.zeros((LEXT, D_MODEL), np.float32)
        keep_ext = np.zeros((LEXT,), np.float32)
        x_ext[lo - (s - H):hi - (s - H)] = x[b, lo:hi]
        keep_ext[lo - (s - H):hi - (s - H)] = 1.0 - mask[b, lo:hi]
        m = dict(shared)
        m["x_ext"] = x_ext
        m["x_mainT"] = np.ascontiguousarray(x[b, s:s + LM].T)
        cc = c32.copy()
        cc[:, 1:1 + LEXT // 128] = keep_ext.reshape(LEXT // 128, 128).T
        m["cst32"] = cc
        c16c = c16s.copy()
        c16c[:, 129 + 2 * D_MODEL:] = keep_ext[H:H + LM].astype(np.float16)[None, :]
        m["cst16"] = c16c
        in_maps.append(m)
    return in_maps


def kernel(**inputs):
    from concourse.bass_utils import run_bass_kernel_spmd
    nc = _get_program()
    in_maps = _prep_inputs(inputs)
    res = run_bass_kernel_spmd(nc, in_maps, core_ids=list(range(8)))
    out = np.zeros((B, L, D_MODEL), np.float32)
    for core in range(8):
        b, half = divmod(core, 2)
        out[b, half * LM:(half + 1) * LM] = res.results[core]["outT"].T
    return out
